# revision 3
# baseline (speedup 1.0000x reference)
"""Windowed correlation (cost volume) kernel for Trainium2, 8 NeuronCores.

Problem: feature1, feature2 (8, 128, 128, 256) fp32 -> out (8, 81, 128, 256),
out[b, ki*9+kj, y, x] = (1/128) * sum_c f1[b,c,y,x] * f2pad[b,c,y+ki,x+kj].

Strategy:
  - Data-parallel over batch: core i handles batch i (c=128 lands on the 128
    SBUF partitions; contraction over c runs on the TensorEngine).
  - Host marshals inputs: f1 is im2col-packed per (8y x 16x) pixel block and
    pre-cast to bf16; f2 is zero-padded (halo 4) and pre-cast to bf16 so every
    device DMA is a plain HWDGE copy.
  - Per pixel block, one bf16 matmul with lhsT = f1 block [c, 128pix] and
    rhs = the padded f2 halo block [c, 16*24=384] computes all pixel-pair
    products; the 81 useful products per pixel sit on diagonals. rhs blocks
    are im2col-staged per y0 row by one ACT copy (single-free-dim operands).
  - The diagonal (shear) extraction is NOT done on device: no engine can
    apply a per-partition offset, and shear-gather DMAs degenerate into
    18-byte descriptors. Instead the full [128pix, 384] slabs are stored
    densely to DRAM with line-rate DMAs (12KB contiguous per partition)
    and the host extracts the 81 diagonals per pixel with a strided view.
  - Loads are chunked (f2 in 17 row-chunks, f1 in 16 block-chunks) and
    prefetched ahead of the row pipeline so DMA stays busy end to end.

Engine plan per y0 row (pipelined):
  SP(sync) prefetch f2 chunk y0+3 / f1 chunk y0+1, then dense store of y0
  ACT      f2row im2col copy (y0)
  PE       16 matmuls (y0) into 4 rotating PSUM banks
  DVE      16 psum->stage copies with 1/128 scale + bf16 cast (y0)
"""

import numpy as np

_B, _C, _H, _W = 8, 128, 128, 256
_K = 9            # kernel size (2*max_disp+1)
_ND = _K * _K     # 81 displacements
_BY, _BX = 8, 16  # pixel block (M = _BY*_BX = 128 = PE rows)
_NBY, _NBX = _H // _BY, _W // _BX        # 16 x 16 blocks
_NA, _NB = _BY + _K - 1, _BX + _K - 1    # 16 x 24 halo block
_NCOLS = _NA * _NB                       # 384 psum columns
_HP, _WP = _H + _K - 1, _W + _K - 1      # padded f2 dims (136, 264)
_NPS = 4                                 # rotating psum banks
_F2CH = _HP // 8                         # 17 f2 row-chunks of 8 rows

_CACHE = {}


def _build_nc():
    from contextlib import ExitStack

    import concourse.bass as bass
    import concourse.mybir as mybir

    nc = bass.Bass()
    # f1 comes in host-packed: [c, y0, x0*128 + ry*16 + rx] bf16
    f1 = nc.dram_tensor(
        "f1", [_C, _NBY, _NBX * 128], mybir.dt.bfloat16, kind="ExternalInput"
    )
    f2 = nc.dram_tensor("f2", [_C, _HP, _WP], mybir.dt.bfloat16, kind="ExternalInput")
    # dense psum slabs: [y0, m, x0, n] (host extracts the diagonals)
    out = nc.dram_tensor(
        "out", [_NBY, 128, _NBX, _NCOLS], mybir.dt.bfloat16, kind="ExternalOutput"
    )

    inv_c = 1.0 / _C
    rows = _NBY
    with ExitStack() as ctx:
        f1blk = ctx.enter_context(
            nc.sbuf_tensor([_C, _NBY * _NBX * 128], mybir.dt.bfloat16)
        )
        f2p = ctx.enter_context(nc.sbuf_tensor([_C, _HP * _WP], mybir.dt.bfloat16))
        f2row = [
            ctx.enter_context(
                nc.sbuf_tensor(f"f2r{i}", [_C, _NBX * _NCOLS], mybir.dt.bfloat16)
            )
            for i in range(2)
        ]
        stage = [
            ctx.enter_context(
                nc.sbuf_tensor(f"stg{i}", [_C, _NBX * _NCOLS], mybir.dt.bfloat16)
            )
            for i in range(2)
        ]
        psum = [
            ctx.enter_context(
                nc.psum_tensor(f"ps{i}", [128, _NCOLS], mybir.dt.float32)
            )
            for i in range(_NPS)
        ]
        s_f1 = ctx.enter_context(nc.semaphore(name="s_f1"))    # +16 per f1 chunk
        s_f2 = ctx.enter_context(nc.semaphore(name="s_f2"))    # +16 per f2 chunk
        s_act = ctx.enter_context(nc.semaphore(name="s_act"))  # +1 per f2row copy
        s_pe = ctx.enter_context(nc.semaphore(name="s_pe"))    # +1 per matmul
        s_dve = ctx.enter_context(nc.semaphore(name="s_dve"))  # +1 per stage copy
        s_st = ctx.enter_context(nc.semaphore(name="s_st"))    # +16 per store
        blk = ctx.enter_context(nc.Block())

        def load_f2_chunk(eng, j):
            src = bass.AP(
                tensor=f2,
                offset=j * 8 * _WP,
                ap=[[_HP * _WP, _C], [1, 8 * _WP]],
            )
            eng.dma_start(f2p[:, j * 8 * _WP : (j + 1) * 8 * _WP], src).then_inc(
                s_f2, 16
            )

        def load_f1_chunk(eng, j):
            src = bass.AP(
                tensor=f1,
                offset=j * _NBX * 128,
                ap=[[_NBY * _NBX * 128, _C], [1, _NBX * 128]],
            )
            eng.dma_start(
                f1blk[:, j * _NBX * 128 : (j + 1) * _NBX * 128], src
            ).then_inc(s_f1, 16)

        @blk.sync
        def _(sync):
            for r in range(rows):
                # store row r once its 16 stage copies are done
                sync.wait_ge(s_dve, (r + 1) * _NBX)
                dst = bass.AP(
                    tensor=out,
                    offset=r * 128 * _NBX * _NCOLS,
                    ap=[[_NBX * _NCOLS, _C], [1, _NBX * _NCOLS]],
                )
                sync.dma_start(dst, stage[r % 2][:, :]).then_inc(s_st, 16)
            sync.wait_ge(s_st, rows * 16)

        @blk.scalar
        def _(scalar):
            # loads ride the scalar HWDGE ring so they don't serialize
            # against the stores on the sync ring.
            # prefetch: f2 chunks 0-2 (row 0 needs 0,1), f1 chunk 0
            load_f2_chunk(scalar, 0)
            load_f2_chunk(scalar, 1)
            load_f1_chunk(scalar, 0)
            load_f2_chunk(scalar, 2)
            for r in range(rows):
                y0 = r
                if r + 3 < _F2CH:
                    load_f2_chunk(scalar, r + 3)
                if r + 1 < rows:
                    load_f1_chunk(scalar, r + 1)
                # f2row needs f2p rows [8r, 8r+16) = chunks r, r+1
                scalar.wait_ge(s_f2, (r + 2) * 16)
                # WAR: matmuls of r-2 read this f2row buffer
                if r >= 2:
                    scalar.wait_ge(s_pe, (r - 1) * _NBX)
                src2 = bass.AP(
                    tensor=f2p,
                    offset=y0 * _BY * _WP,
                    ap=[
                        [_HP * _WP, _C],
                        [_BX, _NBX],
                        [_WP, _NA],
                        [1, _NB],
                    ],
                )
                nc.scalar.activation(
                    f2row[r % 2][:, :], src2, mybir.ActivationFunctionType.Copy
                ).then_inc(s_act, 1)

        @blk.tensor
        def _(tensor):
            for r in range(rows):
                y0 = r
                tensor.wait_ge(s_f1, (r + 1) * 16)
                tensor.wait_ge(s_act, r + 1)
                for x0 in range(_NBX):
                    n = r * _NBX + x0
                    if n >= _NPS:  # WAR: stage copy freed this psum bank
                        tensor.wait_ge(s_dve, n - _NPS + 1)
                    lhsT = f1blk[
                        :, (y0 * _NBX + x0) * 128 : (y0 * _NBX + x0 + 1) * 128
                    ]
                    rhs = f2row[r % 2][:, x0 * _NCOLS : (x0 + 1) * _NCOLS]
                    nc.tensor.matmul(
                        psum[n % _NPS][:, :], lhsT, rhs, start=True, stop=True
                    ).then_inc(s_pe, 1)

        @blk.vector
        def _(vector):
            for r in range(rows):
                # WAR: store of r-2 read this stage buffer
                if r >= 2:
                    vector.wait_ge(s_st, (r - 1) * 16)
                for x0 in range(_NBX):
                    n = r * _NBX + x0
                    vector.wait_ge(s_pe, n + 1)
                    st = stage[r % 2][:, x0 * _NCOLS : (x0 + 1) * _NCOLS]
                    nc.vector.tensor_scalar_mul(
                        st, psum[n % _NPS][:, :], inv_c
                    ).then_inc(s_dve, 1)

    return nc


def _pack_f1(f1_core: np.ndarray) -> np.ndarray:
    """[c, h, w] fp32 -> [c, y0, x0*128 + ry*16 + rx] bf16."""
    import ml_dtypes

    v = f1_core.reshape(_C, _NBY, _BY, _NBX, _BX)
    v = v.transpose(0, 1, 3, 2, 4)  # c, y0, x0, ry, rx
    return np.ascontiguousarray(v.reshape(_C, _NBY, _NBX * 128)).astype(
        ml_dtypes.bfloat16
    )


def _pack_f2(f2_core: np.ndarray) -> np.ndarray:
    """[c, h, w] fp32 -> zero-padded [c, 136, 264] bf16."""
    import ml_dtypes

    f2p = np.zeros((_C, _HP, _WP), dtype=ml_dtypes.bfloat16)
    f2p[:, 4 : 4 + _H, 4 : 4 + _W] = f2_core.astype(ml_dtypes.bfloat16)
    return f2p


def _prep_in_maps(f1: np.ndarray, f2: np.ndarray) -> list:
    return [{"f1": _pack_f1(f1[i]), "f2": _pack_f2(f2[i])} for i in range(_B)]


def _unpack_out(raw: np.ndarray) -> np.ndarray:
    """Dense slab [y0, m, x0, n] bf16 -> [81, h, w] fp32 (diagonal extraction).

    useful element (y0, ry, x0, rx, ki, kj) lives at
    [y0, ry*16+rx, x0, (ry+ki)*24 + rx+kj].
    """
    import ml_dtypes
    from numpy.lib.stride_tricks import as_strided

    u = np.ascontiguousarray(raw).view(np.uint16).reshape(_NBY, 128, _NBX, _NCOLS)
    e = u.itemsize
    # element strides: y0: 128*16*384, m: 16*384, x0: 384, n: 1
    g = as_strided(
        u,
        shape=(_NBY, _BY, _NBX, _BX, _K, _K),
        strides=(
            128 * _NBX * _NCOLS * e,     # y0
            (_BX * _NBX * _NCOLS + _NB) * e,  # ry: m += 16, n += 24
            _NCOLS * e,                  # x0
            (_NBX * _NCOLS + 1) * e,     # rx: m += 1, n += 1
            _NB * e,                     # ki: n += 24
            1 * e,                       # kj: n += 1
        ),
    )
    # -> [ki, kj, y0, ry, x0, rx] -> (81, 128, 256)
    dense = np.ascontiguousarray(g.transpose(4, 5, 0, 1, 2, 3)).reshape(
        _ND, _H, _W
    )
    return dense.view(ml_dtypes.bfloat16).astype(np.float32)


def kernel(feature1: np.ndarray, feature2: np.ndarray) -> np.ndarray:
    from concourse.bass_utils import run_bass_kernel_spmd

    if "nc" not in _CACHE:
        _CACHE["nc"] = _build_nc()
    nc = _CACHE["nc"]

    f1 = np.ascontiguousarray(np.asarray(feature1), dtype=np.float32)
    f2 = np.ascontiguousarray(np.asarray(feature2), dtype=np.float32)
    in_maps = _prep_in_maps(f1, f2)
    res = run_bass_kernel_spmd(nc, in_maps, core_ids=list(range(_B)))
    out = np.stack([_unpack_out(res.results[i]["out"]) for i in range(_B)], axis=0)
    return out


# revision 4
# speedup vs baseline: 1.3147x; 1.3147x over previous
"""Windowed correlation (cost volume) kernel for Trainium2, 8 NeuronCores.

Problem: feature1, feature2 (8, 128, 128, 256) fp32 -> out (8, 81, 128, 256),
out[b, ki*9+kj, y, x] = (1/128) * sum_c f1[b,c,y,x] * f2pad[b,c,y+ki,x+kj].

Strategy:
  - Data-parallel over batch: core i handles batch i (c=128 lands on the 128
    SBUF partitions; contraction over c runs on the TensorEngine).
  - Host marshals inputs: f1 is im2col-packed per (8y x 16x) pixel block and
    pre-cast to bf16; f2 is zero-padded (halo 4) and pre-cast to bf16 so every
    device DMA is a plain HWDGE copy.
  - Per pixel block, one bf16 matmul with lhsT = f1 block [c, 128pix] and
    rhs = the padded f2 halo block [c, 16*24=384] computes all pixel-pair
    products; the 81 useful products per pixel sit on diagonals. The rhs is
    read straight from the resident f2p with a 3-dim AP (no im2col staging).
  - The diagonal (shear) extraction is NOT done on device: no engine can
    apply a per-partition offset (BIR verifier rejects partition-crossing
    SBUF strides), and shear-gather DMAs degenerate into 18-byte
    descriptors. Instead the full [128pix, 384] slabs are stored densely
    to DRAM with line-rate DMAs (12KB contiguous per partition) and the
    host extracts the 81 diagonals per pixel with a strided view (and
    applies the 1/128 scale during the fp32 conversion).
  - PSUM->SBUF copies are split between DVE (even x0) and ACT (odd x0) so
    neither engine paces the pipeline.
  - Loads are chunked and prefetched ahead: f2 (17 row-chunks) on the
    scalar HWDGE ring, f1 (16 block-chunks) on the sync ring alongside the
    stores, keeping both rings and the DMA fabric busy end to end.

Engine plan per y0 row (pipelined):
  SP(sync) prefetch f1 chunk y0+2, two half-row dense stores of y0
  ACT      prefetch f2 chunk y0+3, 8 psum->stage copies (odd x0)
  PE       16 matmuls (y0) into 4 rotating PSUM banks
  DVE      8 psum->stage copies (even x0)
"""

import numpy as np

_B, _C, _H, _W = 8, 128, 128, 256
_K = 9            # kernel size (2*max_disp+1)
_ND = _K * _K     # 81 displacements
_BY, _BX = 8, 16  # pixel block (M = _BY*_BX = 128 = PE rows)
_NBY, _NBX = _H // _BY, _W // _BX        # 16 x 16 blocks
_NA, _NB = _BY + _K - 1, _BX + _K - 1    # 16 x 24 halo block
_NCOLS = _NA * _NB                       # 384 psum columns
_HP, _WP = _H + _K - 1, _W + _K - 1      # padded f2 dims (136, 264)
_NPS = 4                                 # rotating psum banks
_F2CH = _HP // 8                         # 17 f2 row-chunks of 8 rows

_CACHE = {}


def _build_nc():
    from contextlib import ExitStack

    import concourse.bass as bass
    import concourse.mybir as mybir

    nc = bass.Bass()
    # f1 comes in host-packed: [c, y0, x0*128 + ry*16 + rx] bf16
    f1 = nc.dram_tensor(
        "f1", [_C, _NBY, _NBX * 128], mybir.dt.bfloat16, kind="ExternalInput"
    )
    f2 = nc.dram_tensor("f2", [_C, _HP, _WP], mybir.dt.bfloat16, kind="ExternalInput")
    # dense psum slabs: [y0, m, x0, n] (host extracts the diagonals)
    out = nc.dram_tensor(
        "out", [_NBY, 128, _NBX, _NCOLS], mybir.dt.bfloat16, kind="ExternalOutput"
    )

    rows = _NBY
    with ExitStack() as ctx:
        f1blk = ctx.enter_context(
            nc.sbuf_tensor([_C, _NBY * _NBX * 128], mybir.dt.bfloat16)
        )
        f2p = ctx.enter_context(nc.sbuf_tensor([_C, _HP * _WP], mybir.dt.bfloat16))
        stage = [
            ctx.enter_context(
                nc.sbuf_tensor(f"stg{i}", [_C, _NBX * _NCOLS], mybir.dt.bfloat16)
            )
            for i in range(2)
        ]
        psum = [
            ctx.enter_context(
                nc.psum_tensor(f"ps{i}", [128, _NCOLS], mybir.dt.float32)
            )
            for i in range(_NPS)
        ]
        s_f1 = ctx.enter_context(nc.semaphore(name="s_f1"))    # +16 per f1 chunk
        s_f2 = ctx.enter_context(nc.semaphore(name="s_f2"))    # +16 per f2 chunk
        s_pe = ctx.enter_context(nc.semaphore(name="s_pe"))    # +1 per matmul
        s_dve = ctx.enter_context(nc.semaphore(name="s_dve"))  # +1 per DVE copy
        s_sc = ctx.enter_context(nc.semaphore(name="s_sc"))    # +1 per ACT copy
        s_st = ctx.enter_context(nc.semaphore(name="s_st"))    # +16 per half store
        blk = ctx.enter_context(nc.Block())

        def load_f2_chunk(eng, j):
            src = bass.AP(
                tensor=f2,
                offset=j * 8 * _WP,
                ap=[[_HP * _WP, _C], [1, 8 * _WP]],
            )
            eng.dma_start(f2p[:, j * 8 * _WP : (j + 1) * 8 * _WP], src).then_inc(
                s_f2, 16
            )

        def load_f1_chunk(eng, j):
            src = bass.AP(
                tensor=f1,
                offset=j * _NBX * 128,
                ap=[[_NBY * _NBX * 128, _C], [1, _NBX * 128]],
            )
            eng.dma_start(
                f1blk[:, j * _NBX * 128 : (j + 1) * _NBX * 128], src
            ).then_inc(s_f1, 16)

        def copy_sem_wait(eng, n):
            # WAR on psum bank n % 4: its previous user n-4 must be copied out.
            # Copies alternate DVE (even) / ACT (odd); n-4 has n's parity.
            if n >= _NPS:
                m = n - _NPS
                eng.wait_ge(s_dve if m % 2 == 0 else s_sc, m // 2 + 1)

        @blk.sync
        def _(sync):
            load_f1_chunk(sync, 0)
            load_f1_chunk(sync, 1)
            for r in range(rows):
                if r + 2 < rows:
                    load_f1_chunk(sync, r + 2)
                # half-row stores once the 8 covering stage copies are done
                for h in range(2):
                    sync.wait_ge(s_dve, r * 8 + 4 * (h + 1))
                    sync.wait_ge(s_sc, r * 8 + 4 * (h + 1))
                    half = _NBX * _NCOLS // 2
                    dst = bass.AP(
                        tensor=out,
                        offset=r * 128 * _NBX * _NCOLS + h * half,
                        ap=[[_NBX * _NCOLS, _C], [1, half]],
                    )
                    sync.dma_start(
                        dst, stage[r % 2][:, h * half : (h + 1) * half]
                    ).then_inc(s_st, 16)
            sync.wait_ge(s_st, rows * 32)

        @blk.scalar
        def _(scalar):
            # f2 loads ride the scalar HWDGE ring; f1/stores ride sync's.
            load_f2_chunk(scalar, 0)
            load_f2_chunk(scalar, 1)
            load_f2_chunk(scalar, 2)
            load_f2_chunk(scalar, 3)
            for r in range(rows):
                if r + 4 < _F2CH:
                    load_f2_chunk(scalar, r + 4)
                # WAR: stores of r-2 read this stage buffer
                if r >= 2:
                    scalar.wait_ge(s_st, (r - 1) * 32)
                for x0 in range(1, _NBX, 2):
                    n = r * _NBX + x0
                    scalar.wait_ge(s_pe, n + 1)
                    st = stage[r % 2][:, x0 * _NCOLS : (x0 + 1) * _NCOLS]
                    nc.scalar.activation(
                        st, psum[n % _NPS][:, :], mybir.ActivationFunctionType.Copy
                    ).then_inc(s_sc, 1)

        @blk.tensor
        def _(tensor):
            for r in range(rows):
                tensor.wait_ge(s_f1, (r + 1) * 16)
                # matmuls read f2p rows [8r, 8r+16) = chunks r, r+1
                tensor.wait_ge(s_f2, (r + 2) * 16)
                for x0 in range(_NBX):
                    n = r * _NBX + x0
                    copy_sem_wait(tensor, n)
                    lhsT = f1blk[
                        :, (r * _NBX + x0) * 128 : (r * _NBX + x0 + 1) * 128
                    ]
                    rhs = bass.AP(
                        tensor=f2p,
                        offset=r * _BY * _WP + x0 * _BX,
                        ap=[[_HP * _WP, _C], [_WP, _NA], [1, _NB]],
                    )
                    nc.tensor.matmul(
                        psum[n % _NPS][:, :], lhsT, rhs, start=True, stop=True
                    ).then_inc(s_pe, 1)

        @blk.vector
        def _(vector):
            for r in range(rows):
                # WAR: stores of r-2 read this stage buffer
                if r >= 2:
                    vector.wait_ge(s_st, (r - 1) * 32)
                for x0 in range(0, _NBX, 2):
                    n = r * _NBX + x0
                    vector.wait_ge(s_pe, n + 1)
                    st = stage[r % 2][:, x0 * _NCOLS : (x0 + 1) * _NCOLS]
                    nc.vector.tensor_copy(st, psum[n % _NPS][:, :]).then_inc(
                        s_dve, 1
                    )

    return nc


def _pack_f1(f1_core: np.ndarray) -> np.ndarray:
    """[c, h, w] fp32 -> [c, y0, x0*128 + ry*16 + rx] bf16."""
    import ml_dtypes

    v = f1_core.reshape(_C, _NBY, _BY, _NBX, _BX)
    v = v.transpose(0, 1, 3, 2, 4)  # c, y0, x0, ry, rx
    return np.ascontiguousarray(v.reshape(_C, _NBY, _NBX * 128)).astype(
        ml_dtypes.bfloat16
    )


def _pack_f2(f2_core: np.ndarray) -> np.ndarray:
    """[c, h, w] fp32 -> zero-padded [c, 136, 264] bf16."""
    import ml_dtypes

    f2p = np.zeros((_C, _HP, _WP), dtype=ml_dtypes.bfloat16)
    f2p[:, 4 : 4 + _H, 4 : 4 + _W] = f2_core.astype(ml_dtypes.bfloat16)
    return f2p


def _prep_in_maps(f1: np.ndarray, f2: np.ndarray) -> list:
    return [{"f1": _pack_f1(f1[i]), "f2": _pack_f2(f2[i])} for i in range(_B)]


def _unpack_out(raw: np.ndarray) -> np.ndarray:
    """Dense slab [y0, m, x0, n] bf16 -> [81, h, w] fp32 (diagonal extraction).

    useful element (y0, ry, x0, rx, ki, kj) lives at
    [y0, ry*16+rx, x0, (ry+ki)*24 + rx+kj]. Applies the deferred 1/c scale.
    """
    import ml_dtypes
    from numpy.lib.stride_tricks import as_strided

    u = np.ascontiguousarray(raw).view(np.uint16).reshape(_NBY, 128, _NBX, _NCOLS)
    e = u.itemsize
    # element strides: y0: 128*16*384, m: 16*384, x0: 384, n: 1
    g = as_strided(
        u,
        shape=(_NBY, _BY, _NBX, _BX, _K, _K),
        strides=(
            128 * _NBX * _NCOLS * e,     # y0
            (_BX * _NBX * _NCOLS + _NB) * e,  # ry: m += 16, n += 24
            _NCOLS * e,                  # x0
            (_NBX * _NCOLS + 1) * e,     # rx: m += 1, n += 1
            _NB * e,                     # ki: n += 24
            1 * e,                       # kj: n += 1
        ),
    )
    # -> [ki, kj, y0, ry, x0, rx] -> (81, 128, 256)
    dense = np.ascontiguousarray(g.transpose(4, 5, 0, 1, 2, 3)).reshape(
        _ND, _H, _W
    )
    return dense.view(ml_dtypes.bfloat16).astype(np.float32) * (1.0 / _C)


def kernel(feature1: np.ndarray, feature2: np.ndarray) -> np.ndarray:
    from concourse.bass_utils import run_bass_kernel_spmd

    if "nc" not in _CACHE:
        _CACHE["nc"] = _build_nc()
    nc = _CACHE["nc"]

    f1 = np.ascontiguousarray(np.asarray(feature1), dtype=np.float32)
    f2 = np.ascontiguousarray(np.asarray(feature2), dtype=np.float32)
    in_maps = _prep_in_maps(f1, f2)
    res = run_bass_kernel_spmd(nc, in_maps, core_ids=list(range(_B)))
    out = np.stack([_unpack_out(res.results[i]["out"]) for i in range(_B)], axis=0)
    return out
